# revision 2
# baseline (speedup 1.0000x reference)
"""Multi-head attention (B=8, T=1024, D=768, H=12) on 8 TRN2 NeuronCores.

Sharding: data-parallel over batch — one batch element per core, no
collectives. Each core runs the full attention block for its element.

Per-core layout strategy (everything feature-major to avoid on-chip
transposes; host pre-transposes x and the weights):
  xT      [768, 1024]  bf16   (d-major activations)
  wqkvT   [768, 2304]  bf16   (qkv_w.T)
  projT   [768, 768]   bf16   (proj_w.T)
  QKT[j,t] = wqkvT.T @ xT  (+bias)   -> Q.T / K.T per head are row slices
  V[t,jv]  = xT.T @ wqkvT[:,1536:]   (token-major, ones column appended
                                      per head for softmax denominators)
  scoresT[tk,tq] per head: lhsT=K.T-slice [64,128], rhs=Q.T [64,512]
      head pairs packed in PE row groups (partitions 0:64 / 64:128)
  attn = exp(scores * SCALE) on ACT (no max subtraction; |scores|<~40)
  outT[hd+1, tq] = V'.T @ attnT accumulated in PSUM; row 64 = denom
  normalize: DVE reciprocal + gpsimd partition_broadcast + DVE mul
  yT = projT.T @ aoT (+bias'), bias' = proj_b + proj_w @ v_bias
"""

import numpy as np
import ml_dtypes

import concourse.bass as bass
import concourse.mybir as mybir
import concourse.tile as tile
from concourse import bacc
from concourse import bass_utils

BF16 = mybir.dt.bfloat16
F32 = mybir.dt.float32

B, T, D = 8, 1024, 768
H, HD = 12, 64
P = 128
ND = D // P           # 6 d-tiles
NT = T // P           # 8 t-tiles
NPAIR = H // 2        # 6 head pairs
SCALE = HD ** -0.5
HD1 = HD + 1          # V' columns per head (64 V + 1 ones)


def build():
    nc = bacc.Bacc("TRN2", target_bir_lowering=False, debug=False, num_devices=8)

    xT_d = nc.dram_tensor("xT", [D, T], BF16, kind="ExternalInput").ap()
    wqk_d = nc.dram_tensor("wqk", [D, 2 * D], BF16, kind="ExternalInput").ap()
    wv_d = nc.dram_tensor("wv", [D, D], BF16, kind="ExternalInput").ap()
    projT_d = nc.dram_tensor("projT", [D, D], BF16, kind="ExternalInput").ap()
    qkb_d = nc.dram_tensor("qkb", [P, 2 * ND], F32, kind="ExternalInput").ap()
    pb2_d = nc.dram_tensor("pb2", [P, ND], F32, kind="ExternalInput").ap()
    yT_d = nc.dram_tensor("yT", [D, T], F32, kind="ExternalOutput").ap()

    with tile.TileContext(nc) as tc:
        with tc.tile_pool(name="const", bufs=1) as const, \
             tc.tile_pool(name="work", bufs=4) as work, \
             tc.tile_pool(name="norm", bufs=2) as normp, \
             tc.tile_pool(name="yout", bufs=2) as yout, \
             tc.tile_pool(name="psA", bufs=2, space="PSUM") as psA, \
             tc.tile_pool(name="psO", bufs=2, space="PSUM") as psO:

            # ---- resident SBUF tensors ----
            xT_sb = const.tile([P, ND, T], BF16)
            wv_sb = const.tile([P, ND, D], BF16)
            wqk_sb = const.tile([P, ND, 2 * D], BF16)
            projT_sb = const.tile([P, ND, D], BF16)
            qkb_sb = const.tile([P, 2 * ND], F32)
            pb2_sb = const.tile([P, ND], F32)
            QKT_sb = const.tile([P, 2 * ND, T], BF16)
            V_sb = const.tile([P, NT, H * HD1], BF16)
            aoT_sb = const.tile([P, ND, T], BF16)

            nc.sync.dma_start(xT_sb[:], xT_d.rearrange("(ko p) t -> p ko t", p=P))
            nc.sync.dma_start(wv_sb[:], wv_d.rearrange("(ko p) j -> p ko j", p=P))
            nc.sync.dma_start(qkb_sb[:], qkb_d)
            nc.sync.dma_start(wqk_sb[:], wqk_d.rearrange("(ko p) j -> p ko j", p=P))
            nc.sync.dma_start(projT_sb[:], projT_d.rearrange("(ko p) j -> p ko j", p=P))
            nc.sync.dma_start(pb2_sb[:], pb2_d)

            # ones columns for the softmax denominator (col 64 of each head's V')
            nc.vector.memset(V_sb[:], 1.0)

            # warm the exp table set early (one-time ~2.7us table load)
            warm = work.tile([1, 12], F32, tag="warm", bufs=1)
            nc.scalar.activation(warm[:], qkb_sb[0:1, 0:12], mybir.ActivationFunctionType.Exp)

            # ---- V projection: V[t, jv] = xT.T @ wv  (token-major) ----
            for t in range(NT):
                for jc, (j0, jn) in enumerate([(0, 512), (512, 256)]):
                    ps_v = psA.tile([P, T], F32, tag="big", name=f"psv_{t}_{jc}")
                    for d in range(ND):
                        nc.tensor.matmul(
                            ps_v[:, :jn],
                            xT_sb[:, d, t * P:(t + 1) * P],
                            wv_sb[:, d, j0:j0 + jn],
                            start=(d == 0), stop=(d == ND - 1),
                        )
                    nh = jn // HD
                    h0 = j0 // HD
                    dst = V_sb[:, t, :].rearrange("p (h c) -> p h c", c=HD1)
                    nc.vector.tensor_copy(
                        out=dst[:, h0:h0 + nh, 0:HD],
                        in_=ps_v[:, :jn].rearrange("p (h c) -> p h c", c=HD),
                    )

            def emit_qk(jt):
                # QKT[:, jt, :] for j-tile jt (0..5 = Q, 6..11 = K)
                ps_qk = psA.tile([P, T], F32, tag="big", name=f"psqk_{jt}")
                for tq in range(2):
                    for d in range(ND):
                        nc.tensor.matmul(
                            ps_qk[:, tq * 512:(tq + 1) * 512],
                            wqk_sb[:, d, jt * P:(jt + 1) * P],
                            xT_sb[:, d, tq * 512:(tq + 1) * 512],
                            start=(d == 0), stop=(d == ND - 1),
                        )
                nc.vector.tensor_scalar_add(
                    QKT_sb[:, jt, :], ps_qk[:], qkb_sb[:, jt:jt + 1])

            emit_qk(0)
            emit_qk(ND)  # K tile for pair 0

            # ---- attention, one head-pair at a time ----
            for i in range(NPAIR):
                oacc = [psO.tile([HD1, T], F32, tag="oacc", name=f"oacc_{i}_{hh}")
                        for hh in range(2)]
                for tk in range(NT):
                    for hh in range(2):
                        p0 = 64 * hh
                        sc = psA.tile([P, T], F32, tag="big", name=f"sc_{i}_{tk}_{hh}")
                        for tq in range(2):
                            nc.tensor.matmul(
                                sc[:, tq * 512:(tq + 1) * 512],
                                QKT_sb[p0:p0 + 64, ND + i, tk * P:(tk + 1) * P],
                                QKT_sb[p0:p0 + 64, i, tq * 512:(tq + 1) * 512],
                            )
                        at = work.tile([P, T], BF16, tag="attn", name=f"at_{i}_{tk}_{hh}")
                        nc.scalar.activation(
                            at[:], sc[:], mybir.ActivationFunctionType.Exp, scale=SCALE)
                        h = 2 * i + hh
                        for tq in range(2):
                            nc.tensor.matmul(
                                oacc[hh][:, tq * 512:(tq + 1) * 512],
                                V_sb[:, tk, h * HD1:(h + 1) * HD1],
                                at[:, tq * 512:(tq + 1) * 512],
                                start=(tk == 0), stop=(tk == NT - 1),
                            )
                # normalize: aoT[64*hh:+64, i, :] = oacc[0:64] / oacc[64]
                for hh in range(2):
                    rsb = normp.tile([1, T], F32, tag="rsb", name=f"rsb_{i}_{hh}")
                    nc.vector.reciprocal(rsb[:], oacc[hh][HD:HD1, :])
                    rbc = normp.tile([64, T], F32, tag="rbc", name=f"rbc_{i}_{hh}")
                    nc.gpsimd.partition_broadcast(rbc[:], rsb[:])
                    nc.vector.tensor_tensor(
                        aoT_sb[64 * hh:64 * hh + 64, i, :],
                        oacc[hh][0:HD, :], rbc[:], mybir.AluOpType.mult)
                # queue next pair's Q/K projection tiles (PE gap filler)
                if i + 1 < NPAIR:
                    emit_qk(i + 1)
                    emit_qk(ND + i + 1)

            # ---- output projection: yT = projT.T @ aoT (+ pb2) ----
            for dt in range(ND):
                ps_y = psO.tile([P, T], F32, tag="oacc", name=f"psy_{dt}")
                for tq in range(2):
                    for ko in range(ND):
                        nc.tensor.matmul(
                            ps_y[:, tq * 512:(tq + 1) * 512],
                            projT_sb[:, ko, dt * P:(dt + 1) * P],
                            aoT_sb[:, ko, tq * 512:(tq + 1) * 512],
                            start=(ko == 0), stop=(ko == ND - 1),
                        )
                yt = yout.tile([P, T], F32, tag="yt", name=f"yt_{dt}")
                nc.vector.tensor_scalar_add(yt[:], ps_y[:], pb2_sb[:, dt:dt + 1])
                nc.sync.dma_start(yT_d[dt * P:(dt + 1) * P, :], yt[:])

    nc.compile()
    return nc


def prep_inputs(x, qkv_w, qkv_b, proj_w, proj_b):
    """Host-side layout prep. Returns per-core input maps."""
    bf = ml_dtypes.bfloat16
    wqkvT = np.ascontiguousarray(qkv_w.T)          # [768, 2304] f32
    wqk = wqkvT[:, :2 * D].astype(bf)
    wv = np.ascontiguousarray(wqkvT[:, 2 * D:]).astype(bf)
    projT = np.ascontiguousarray(proj_w.T).astype(bf)
    qkb = np.ascontiguousarray(
        qkv_b[:2 * D].reshape(2 * ND, P).T).astype(np.float32)   # [128, 12]
    vb = qkv_b[2 * D:]
    pb2 = (proj_b + proj_w @ vb).astype(np.float32)
    pb2 = np.ascontiguousarray(pb2.reshape(ND, P).T)             # [128, 6]

    in_maps = []
    for b in range(B):
        xT = np.ascontiguousarray(x[b].T).astype(bf)             # [768, 1024]
        in_maps.append({
            "xT": xT, "wqk": wqk, "wv": wv, "projT": projT,
            "qkb": qkb, "pb2": pb2,
        })
    return in_maps


_CACHE = {}


def kernel(x, qkv_w, qkv_b, proj_w, proj_b):
    x = np.asarray(x, dtype=np.float32)
    qkv_w = np.asarray(qkv_w, dtype=np.float32)
    qkv_b = np.asarray(qkv_b, dtype=np.float32)
    proj_w = np.asarray(proj_w, dtype=np.float32)
    proj_b = np.asarray(proj_b, dtype=np.float32)

    if "nc" not in _CACHE:
        _CACHE["nc"] = build()
    nc = _CACHE["nc"]

    in_maps = prep_inputs(x, qkv_w, qkv_b, proj_w, proj_b)
    res = bass_utils.run_bass_kernel_spmd(nc, in_maps, core_ids=list(range(8)))
    out = np.empty((B, T, D), np.float32)
    for b in range(B):
        out[b] = res.results[b]["yT"].T
    return out


if __name__ == "__main__":
    rng = np.random.default_rng(0)
    ins = {
        "x": rng.standard_normal((B, T, D), dtype=np.float32),
        "qkv_w": rng.standard_normal((3 * D, D), dtype=np.float32) * D ** -0.5,
        "qkv_b": rng.standard_normal(3 * D).astype(np.float32) * 0.02,
        "proj_w": rng.standard_normal((D, D), dtype=np.float32) * D ** -0.5,
        "proj_b": rng.standard_normal(D).astype(np.float32) * 0.02,
    }
    out = kernel(**ins)
    print("ok", out.shape, np.abs(out).max())


# revision 4
# speedup vs baseline: 1.2477x; 1.2477x over previous
"""Multi-head attention (B=8, T=1024, D=768, H=12) on 8 TRN2 NeuronCores.

Sharding: data-parallel over batch — one batch element per core, no
collectives. Each core runs the full attention block for its element.

Per-core layout strategy (everything feature-major to avoid on-chip
transposes; host pre-transposes x and the weights):
  xT      [768, 1024]  bf16   (d-major activations)
  wqkvT   [768, 2304]  bf16   (qkv_w.T)
  projT   [768, 768]   bf16   (proj_w.T)
  QKT[j,t] = wqkvT.T @ xT  (+bias)   -> Q.T / K.T per head are row slices
  V[t,jv]  = xT.T @ wqkvT[:,1536:]   (token-major, ones column appended
                                      per head for softmax denominators)
  scoresT[tk,tq] per head: lhsT=K.T-slice [64,128], rhs=Q.T [64,512]
      head pairs packed in PE row groups (partitions 0:64 / 64:128)
  attn = exp(scores * SCALE) on ACT (no max subtraction; |scores|<~40)
  outT[hd+1, tq] = V'.T @ attnT accumulated in PSUM; row 64 = denom
  normalize: DVE reciprocal + gpsimd partition_broadcast + DVE mul
  yT = projT.T @ aoT (+bias'), bias' = proj_b + proj_w @ v_bias
"""

import numpy as np
import ml_dtypes

import concourse.bass as bass
import concourse.mybir as mybir
import concourse.tile as tile
from concourse import bacc
from concourse import bass_utils

BF16 = mybir.dt.bfloat16
F32 = mybir.dt.float32

B, T, D = 8, 1024, 768
H, HD = 12, 64
P = 128
ND = D // P           # 6 d-tiles
NT = T // P           # 8 t-tiles
NPAIR = H // 2        # 6 head pairs
SCALE = HD ** -0.5
HD1 = HD + 1          # V' columns per head (64 V + 1 ones)


def build():
    nc = bacc.Bacc("TRN2", target_bir_lowering=False, debug=False, num_devices=8)

    xT_d = nc.dram_tensor("xT", [D, T], BF16, kind="ExternalInput").ap()
    wqk_d = nc.dram_tensor("wqk", [D, 2 * D], BF16, kind="ExternalInput").ap()
    wv_d = nc.dram_tensor("wv", [D, D], BF16, kind="ExternalInput").ap()
    projT_d = nc.dram_tensor("projT", [D, D], BF16, kind="ExternalInput").ap()
    qkb_d = nc.dram_tensor("qkb", [P, 2 * ND], F32, kind="ExternalInput").ap()
    pb2_d = nc.dram_tensor("pb2", [P, ND], F32, kind="ExternalInput").ap()
    yT_d = nc.dram_tensor("yT", [D, T], F32, kind="ExternalOutput").ap()

    with tile.TileContext(nc) as tc:
        with tc.tile_pool(name="const", bufs=1) as const, \
             tc.tile_pool(name="work", bufs=4) as work, \
             tc.tile_pool(name="norm", bufs=2) as normp, \
             tc.tile_pool(name="yout", bufs=2) as yout, \
             tc.tile_pool(name="psA", bufs=2, space="PSUM") as psA, \
             tc.tile_pool(name="psO", bufs=2, space="PSUM") as psO:

            # ---- resident SBUF tensors ----
            xT_sb = const.tile([P, ND, T], BF16)
            wv_sb = const.tile([P, ND, D], BF16)
            wqk_sb = const.tile([P, ND, 2 * D], BF16)
            projT_sb = const.tile([P, ND, D], BF16)
            qkb_sb = const.tile([P, 2 * ND], F32)
            pb2_sb = const.tile([P, ND], F32)
            QKT_sb = const.tile([P, 2 * ND, T], BF16)
            V_sb = const.tile([P, NT, H * HD1], BF16)
            aoT_sb = const.tile([P, ND, T], BF16)

            nc.sync.dma_start(xT_sb[:], xT_d.rearrange("(ko p) t -> p ko t", p=P))
            nc.sync.dma_start(wv_sb[:], wv_d.rearrange("(ko p) j -> p ko j", p=P))
            nc.sync.dma_start(qkb_sb[:], qkb_d)
            nc.sync.dma_start(wqk_sb[:], wqk_d.rearrange("(ko p) j -> p ko j", p=P))
            nc.sync.dma_start(projT_sb[:], projT_d.rearrange("(ko p) j -> p ko j", p=P))
            nc.sync.dma_start(pb2_sb[:], pb2_d)

            # ones columns for the softmax denominator (col 64 of each head's V')
            nc.vector.memset(V_sb[:], 1.0)

            # warm the exp table set early (one-time ~2.7us table load)
            warm = work.tile([1, 12], F32, tag="warm", bufs=1)
            nc.scalar.activation(warm[:], qkb_sb[0:1, 0:12], mybir.ActivationFunctionType.Exp)

            # ---- V projection: V[t, jv] = xT.T @ wv  (token-major) ----
            for t in range(NT):
                for jc, (j0, jn) in enumerate([(0, 512), (512, 256)]):
                    ps_v = psA.tile([P, T], F32, tag="big", name=f"psv_{t}_{jc}")
                    for d in range(ND):
                        nc.tensor.matmul(
                            ps_v[:, :jn],
                            xT_sb[:, d, t * P:(t + 1) * P],
                            wv_sb[:, d, j0:j0 + jn],
                            start=(d == 0), stop=(d == ND - 1),
                        )
                    nh = jn // HD
                    h0 = j0 // HD
                    dst = V_sb[:, t, :].rearrange("p (h c) -> p h c", c=HD1)
                    nc.vector.tensor_copy(
                        out=dst[:, h0:h0 + nh, 0:HD],
                        in_=ps_v[:, :jn].rearrange("p (h c) -> p h c", c=HD),
                    )

            def emit_qk(jt):
                # QKT[:, jt, :] for j-tile jt (0..5 = Q, 6..11 = K)
                ps_qk = psA.tile([P, T], F32, tag="big", name=f"psqk_{jt}")
                for tq in range(2):
                    for d in range(ND):
                        nc.tensor.matmul(
                            ps_qk[:, tq * 512:(tq + 1) * 512],
                            wqk_sb[:, d, jt * P:(jt + 1) * P],
                            xT_sb[:, d, tq * 512:(tq + 1) * 512],
                            start=(d == 0), stop=(d == ND - 1),
                        )
                nc.vector.tensor_scalar_add(
                    QKT_sb[:, jt, :], ps_qk[:], qkb_sb[:, jt:jt + 1])

            emit_qk(0)
            emit_qk(ND)  # K tile for pair 0

            # ---- attention, one head-pair at a time ----
            for i in range(NPAIR):
                oacc = [psO.tile([HD1, T], F32, tag="oacc", name=f"oacc_{i}_{hh}")
                        for hh in range(2)]
                for tk in range(NT):
                    for hh in range(2):
                        p0 = 64 * hh
                        sc = psA.tile([P, T], F32, tag="big", name=f"sc_{i}_{tk}_{hh}")
                        for tq in range(2):
                            nc.tensor.matmul(
                                sc[:, tq * 512:(tq + 1) * 512],
                                QKT_sb[p0:p0 + 64, ND + i, tk * P:(tk + 1) * P],
                                QKT_sb[p0:p0 + 64, i, tq * 512:(tq + 1) * 512],
                            )
                        at = work.tile([P, T], BF16, tag="attn", name=f"at_{i}_{tk}_{hh}")
                        nc.scalar.activation(
                            at[:], sc[:], mybir.ActivationFunctionType.Exp, scale=SCALE)
                        h = 2 * i + hh
                        for tq in range(2):
                            nc.tensor.matmul(
                                oacc[hh][:, tq * 512:(tq + 1) * 512],
                                V_sb[:, tk, h * HD1:(h + 1) * HD1],
                                at[:, tq * 512:(tq + 1) * 512],
                                start=(tk == 0), stop=(tk == NT - 1),
                            )
                # queue next pair's Q/K projection tiles (PE gap filler at the
                # pair boundary, while the normalize chain below runs on
                # DVE/GpSimd)
                if i + 1 < NPAIR:
                    emit_qk(i + 1)
                    emit_qk(ND + i + 1)
                # normalize: aoT[64*hh:+64, i, :] = oacc[0:64] / oacc[64]
                for hh in range(2):
                    den = normp.tile([1, T], F32, tag="den", name=f"den_{i}_{hh}")
                    nc.vector.tensor_copy(out=den[:], in_=oacc[hh][HD:HD1, :])
                    rsb = normp.tile([1, T], F32, tag="rsb", name=f"rsb_{i}_{hh}")
                    nc.vector.reciprocal_approx_fast(rsb[:], den[:])
                    rbc = normp.tile([64, T], F32, tag="rbc", name=f"rbc_{i}_{hh}")
                    nc.gpsimd.partition_broadcast(rbc[:], rsb[:])
                    nc.vector.tensor_tensor(
                        aoT_sb[64 * hh:64 * hh + 64, i, :],
                        oacc[hh][0:HD, :], rbc[:], mybir.AluOpType.mult)

            # ---- output projection: yT = projT.T @ aoT (+ pb2) ----
            for dt in range(ND):
                ps_y = psO.tile([P, T], F32, tag="oacc", name=f"psy_{dt}")
                for tq in range(2):
                    for ko in range(ND):
                        nc.tensor.matmul(
                            ps_y[:, tq * 512:(tq + 1) * 512],
                            projT_sb[:, ko, dt * P:(dt + 1) * P],
                            aoT_sb[:, ko, tq * 512:(tq + 1) * 512],
                            start=(ko == 0), stop=(ko == ND - 1),
                        )
                yt = yout.tile([P, T], F32, tag="yt", name=f"yt_{dt}")
                nc.vector.tensor_scalar_add(yt[:], ps_y[:], pb2_sb[:, dt:dt + 1])
                nc.sync.dma_start(yT_d[dt * P:(dt + 1) * P, :], yt[:])

    nc.compile()
    return nc


def prep_inputs(x, qkv_w, qkv_b, proj_w, proj_b):
    """Host-side layout prep. Returns per-core input maps."""
    bf = ml_dtypes.bfloat16
    wqkvT = np.ascontiguousarray(qkv_w.T)          # [768, 2304] f32
    wqk = wqkvT[:, :2 * D].astype(bf)
    wv = np.ascontiguousarray(wqkvT[:, 2 * D:]).astype(bf)
    projT = np.ascontiguousarray(proj_w.T).astype(bf)
    qkb = np.ascontiguousarray(
        qkv_b[:2 * D].reshape(2 * ND, P).T).astype(np.float32)   # [128, 12]
    vb = qkv_b[2 * D:]
    pb2 = (proj_b + proj_w @ vb).astype(np.float32)
    pb2 = np.ascontiguousarray(pb2.reshape(ND, P).T)             # [128, 6]

    in_maps = []
    for b in range(B):
        xT = np.ascontiguousarray(x[b].T).astype(bf)             # [768, 1024]
        in_maps.append({
            "xT": xT, "wqk": wqk, "wv": wv, "projT": projT,
            "qkb": qkb, "pb2": pb2,
        })
    return in_maps


_CACHE = {}


def kernel(x, qkv_w, qkv_b, proj_w, proj_b):
    x = np.asarray(x, dtype=np.float32)
    qkv_w = np.asarray(qkv_w, dtype=np.float32)
    qkv_b = np.asarray(qkv_b, dtype=np.float32)
    proj_w = np.asarray(proj_w, dtype=np.float32)
    proj_b = np.asarray(proj_b, dtype=np.float32)

    if "nc" not in _CACHE:
        _CACHE["nc"] = build()
    nc = _CACHE["nc"]

    in_maps = prep_inputs(x, qkv_w, qkv_b, proj_w, proj_b)
    res = bass_utils.run_bass_kernel_spmd(nc, in_maps, core_ids=list(range(8)))
    out = np.empty((B, T, D), np.float32)
    for b in range(B):
        out[b] = res.results[b]["yT"].T
    return out


if __name__ == "__main__":
    rng = np.random.default_rng(0)
    ins = {
        "x": rng.standard_normal((B, T, D), dtype=np.float32),
        "qkv_w": rng.standard_normal((3 * D, D), dtype=np.float32) * D ** -0.5,
        "qkv_b": rng.standard_normal(3 * D).astype(np.float32) * 0.02,
        "proj_w": rng.standard_normal((D, D), dtype=np.float32) * D ** -0.5,
        "proj_b": rng.standard_normal(D).astype(np.float32) * 0.02,
    }
    out = kernel(**ins)
    print("ok", out.shape, np.abs(out).max())
